# revision 15
# baseline (speedup 1.0000x reference)
"""GCNConv custom kernel for Trainium2 (8 NeuronCores, SPMD row-sharded).

Math (matches the reference exactly):
    A = max(scatter(edges), scatter(edges).T) + I        # dense [N, N]
    deg = A.sum(axis=1); d = 1/sqrt(deg + EPS)
    out = (d[:,None] * A * d[None,:]) @ x @ W + b

Device d owns output rows [1024*d, 1024*(d+1)).  The host reformats
edge_index into the dense 0/1/2 adjacency slab for those rows (exact small
integers, fp8e4, laid out [j%128, half, j//128, li%512]) plus integer
degree counts; all floating-point math stays on device:

  z   = rsqrt(deg+eps) * x          (fp16, per-j-tile DVE scale)
  z8  = zhi (fp8e4) + zlo (fp8e5)   (hi/lo split keeps fp8 error ~1e-3)
  aggT[c, li] = sum_j z[j, c] * A_loc[li, j]
      -> DoubleRow PE matmuls (2 j-tiles per instruction, fp8 x fp8,
         0.5 cyc/row), hi+lo passes accumulating into one PSUM group
  aggs = aggT * d_my[li]            (row scale; d_my tiled to [128,1024] by
                                     ones^T @ (ident (.) d_my) matmuls)
  out = aggs^T @ W + b              (per-li-tile matmul; bias rides the
                                     same PSUM group as a rank-1 matmul)

Performance structure: the DMA transfer queues serialize per ISSUING
engine, so the adjacency stream is split across three channels (SP, ACT
HWDGE, Pool SWDGE) and runs in ~1/3 the serial time; x is shipped
host-transposed so its cast-DMA descriptors stay contiguous.  The li
columns are processed in two halves end-to-end: the first half's
row-scale / W-apply / copy-out / store hides under the second half's
matmuls.  All PSUM tiles share one pool (no bank-reuse serialization).
No collectives (degrees are integer graph structure, host-side).
"""

import sys

for _p in ("/root/.axon_site", "/root/.axon_site/_ro/trn_rl_repo", "/opt/trn_rl_repo"):
    if _p not in sys.path:
        sys.path.append(_p)

import ml_dtypes
import numpy as np

import concourse.bass as bass
import concourse.mybir as mybir
import concourse.tile as tile
from concourse import bacc
from concourse import bass_utils
from concourse.masks import make_identity

F32 = mybir.dt.float32
F16 = mybir.dt.float16
F8 = mybir.dt.float8e4
F8L = mybir.dt.float8e5

N = 8192
D = 128
NDEV = 8
NSH = N // NDEV          # rows per device
NT = N // 128            # j tiles
NL = NSH // 128          # li tiles
EPS = 1e-5
BCH = 4                  # j-tiles per adjacency DMA chunk
NCH = NT // BCH          # chunks per li-half
XCH = (4, 20, 20, 20)    # j-tiles per x DMA chunk (small first to unblock z0)
ZG = 4                   # j-tiles per fp8-cast group

# adjacency chunk -> issuing DMA channel (sp / act / pool), tuned so each
# channel's stream finishes just before the matmuls need it
A_ENG = ["sp", "sp", "sp", "sp", "act", "act", "act", "act",
         "sp", "sp", "sp", "sp", "sp", "sp", "sp", "sp"]
B_ENG = ["pool", "pool", "pool", "pool", "pool", "pool", "pool", "pool",
         "act", "act", "act", "act", "sp", "sp", "sp", "sp"]


def _build_program(n=N, d=D, ndev=NDEV):
    """SPMD bass program; all per-core variation arrives as input data."""
    nsh = n // ndev
    nt = n // 128
    nl = nsh // 128
    hw = nsh // 2            # li-half width

    nc = bacc.Bacc("TRN2", target_bir_lowering=False, debug=False,
                   num_devices=ndev)

    xt_d = nc.dram_tensor("xt", [128, nt * d], F32, kind="ExternalInput")
    w_d = nc.dram_tensor("w", [d, d], F32, kind="ExternalInput")
    b_d = nc.dram_tensor("b", [1, d], F32, kind="ExternalInput")
    ablk_d = nc.dram_tensor("ablk", [128, nt * nsh], F8, kind="ExternalInput")
    deg_d = nc.dram_tensor("deg", [128, nt], F32, kind="ExternalInput")
    degmy_d = nc.dram_tensor("degmy", [128, nl], F32, kind="ExternalInput")
    out_d = nc.dram_tensor("out", [nsh, d], F32, kind="ExternalOutput")

    eng = {"sp": nc.sync, "act": nc.scalar, "pool": nc.gpsimd}

    with tile.TileContext(nc) as tc:
        with (
            tc.tile_pool(name="const", bufs=1) as cpool,
            tc.tile_pool(name="blocks", bufs=1) as bpool,
            tc.tile_pool(name="psum", bufs=1, space="PSUM") as ppool,
        ):
            psum_md = ppool.tile([128, nsh], F32)
            psum_agg = ppool.tile([128, nsh], F32)
            psum_o = ppool.tile([128, nl, d], F32)

            # ---- small inputs first (degrees gate the z scaling) ----
            degt = cpool.tile([128, nt], F32)
            nc.sync.dma_start(out=degt[:], in_=deg_d.ap())
            degmy = cpool.tile([128, nl], F32)
            nc.sync.dma_start(out=degmy[:], in_=degmy_d.ap())
            ones1 = cpool.tile([128, d], F16)
            nc.vector.memset(ones1[:], 1.0)
            ident = cpool.tile([128, 128], F16)
            make_identity(nc, ident[:])

            # x (cast f32->fp16 in flight; host pre-transposed to [p, t, c])
            xz = cpool.tile([128, nt, d], F16)
            xv = xt_d.ap().rearrange("p (t c) -> p t c", c=d)
            c0 = 0
            for w_ in XCH:
                nc.gpsimd.dma_start(out=xz[:, c0:c0 + w_, :],
                                    in_=xv[:, c0:c0 + w_, :])
                c0 += w_
            wt = cpool.tile([128, d], F16)
            nc.gpsimd.dma_start(out=wt[:], in_=w_d.ap())
            brow = cpool.tile([1, d], F16)
            nc.gpsimd.dma_start(out=brow[:], in_=b_d.ap())

            # ---- d = 1/sqrt(deg + eps): recips on DVE, sqrts first in the
            # ACT stream (ahead of its DMA issues) ----
            rect = cpool.tile([128, nt], F32)
            dt_ = cpool.tile([128, nt], F32)
            recmy = cpool.tile([128, nl], F32)
            mydf = cpool.tile([128, nl], F32)
            nc.vector.tensor_scalar_add(degt[:], degt[:], EPS)
            nc.vector.reciprocal(rect[:], degt[:])
            nc.vector.tensor_scalar_add(degmy[:], degmy[:], EPS)
            nc.vector.reciprocal(recmy[:], degmy[:])
            nc.scalar.sqrt(dt_[:], rect[:])
            nc.scalar.sqrt(mydf[:], recmy[:])

            # adjacency slab: [p, half, t, li'] so each half streams densely;
            # chunks spread over the three DMA channels
            blk = bpool.tile([128, 2, nt, hw], F8)
            av = ablk_d.ap().rearrange("p (s t l) -> p s t l", s=2, l=hw)

            def blk_dma(half, ci, engine):
                t0 = ci * BCH
                eng[engine].dma_start(out=blk[:, half, t0:t0 + BCH, :],
                                      in_=av[:, half, t0:t0 + BCH, :])

            for ci in range(NCH):         # half A: act chunks first (their
                if A_ENG[ci] == "act":    # SEQ slots are right after sqrts)
                    blk_dma(0, ci, "act")
            for ci in range(NCH):
                if A_ENG[ci] == "sp":
                    blk_dma(0, ci, "sp")

            # ---- z = d * x (fp16) + hi/lo fp8 split, per ZG-tile group;
            # the d_my broadcast build rides the gaps after group 3 ----
            zhi = cpool.tile([128, nt, d], F8)
            zlo = cpool.tile([128, nt, d], F8L)
            diagm = cpool.tile([128, nl, 128], F16)
            mydbc = cpool.tile([128, nsh], F32)
            for g0 in range(0, nt, ZG):
                for t in range(g0, g0 + ZG):
                    nc.vector.tensor_scalar_mul(
                        xz[:, t, :], xz[:, t, :], dt_[:, t:t + 1])
                nc.scalar.activation(
                    out=zhi[:, g0:g0 + ZG, :], in_=xz[:, g0:g0 + ZG, :],
                    func=mybir.ActivationFunctionType.Copy)
                nc.vector.tensor_tensor(
                    out=zlo[:, g0:g0 + ZG, :], in0=xz[:, g0:g0 + ZG, :],
                    in1=zhi[:, g0:g0 + ZG, :],
                    op=mybir.AluOpType.subtract)
                if g0 == 3 * ZG:
                    # mydbc[c, li] = d_my[li]: ones^T @ (ident (.) d_my_lt)
                    for lt in range(nl):
                        nc.vector.tensor_scalar_mul(
                            diagm[:, lt, :], ident[:], mydf[:, lt:lt + 1])
                    for lt in range(nl):
                        nc.tensor.matmul(
                            out=psum_md[:, lt * 128:(lt + 1) * 128],
                            lhsT=ones1[:], rhs=diagm[:, lt, :],
                            start=True, stop=True)
                    nc.vector.tensor_copy(out=mydbc[:], in_=psum_md[:])

            # ---- aggregation (DoubleRow, 2 j-tiles/mm, hi+lo passes) ----
            ntp = nt // 2

            def agg_mms(half, tps):
                for tp in tps:
                    t0 = 2 * tp
                    for z8 in (zhi, zlo):
                        nc.tensor.matmul(
                            out=psum_agg[:, half * hw:(half + 1) * hw],
                            lhsT=z8[:, t0:t0 + 2, :],
                            rhs=blk[:, half, t0:t0 + 2, :],
                            perf_mode=mybir.MatmulPerfMode.DoubleRow,
                            start=(tp == 0 and z8 is zhi),
                            stop=(tp == ntp - 1 and z8 is zlo))

            aggs = cpool.tile([128, nsh], F16)
            o_all = cpool.tile([128, nl, d], F32)
            ov = out_d.ap().rearrange("(t p) c -> p t c", p=128)

            def tail(half):
                l0 = half * (nl // 2)
                l1 = l0 + nl // 2
                nc.vector.tensor_tensor(
                    out=aggs[:, l0 * 128:l1 * 128],
                    in0=psum_agg[:, l0 * 128:l1 * 128],
                    in1=mydbc[:, l0 * 128:l1 * 128],
                    op=mybir.AluOpType.mult)
                for lt in range(l0, l1):
                    nc.tensor.matmul(
                        out=psum_o[:, lt, :],
                        lhsT=aggs[:, lt * 128:(lt + 1) * 128],
                        rhs=wt[:], start=True, stop=False)
                    nc.tensor.matmul(
                        out=psum_o[:, lt, :],
                        lhsT=ones1[0:1, :], rhs=brow[:],
                        start=False, stop=True)
                nc.scalar.activation(
                    out=o_all[:, l0:l1, :], in_=psum_o[:, l0:l1, :],
                    func=mybir.ActivationFunctionType.Copy)
                nc.sync.dma_start(out=ov[:, l0:l1, :], in_=o_all[:, l0:l1, :])

            agg_mms(0, range(ntp))
            # half-B stream: pool-issued chunks queue behind x/w; act-issued
            # behind the casts; sp-issued behind half A's
            for engine in ("pool", "act", "sp"):
                for ci in range(NCH):
                    if B_ENG[ci] == engine:
                        blk_dma(1, ci, engine)
            # W-apply of half A hides in half B's DMA-chase gaps
            agg_mms(1, range(4))
            tail(0)
            agg_mms(1, range(4, ntp))
            tail(1)

    nc.compile()
    return nc


_F8LUT = np.array([0.0, 1.0, 2.0], dtype=ml_dtypes.float8_e4m3fn).view(np.uint8)


def _host_prep(x, edge_index, weight, bias, n=N, ndev=NDEV):
    """Reformat edge_index into per-device dense fp8 adjacency slabs plus
    integer degree counts (graph structure only; all FP math is on device)."""
    nsh = n // ndev
    nt = n // 128
    nl = nsh // 128
    d = x.shape[1]

    a = np.asarray(edge_index[0], dtype=np.int64)
    b = np.asarray(edge_index[1], dtype=np.int64)

    m = np.zeros((n, n), dtype=np.uint8)
    m[a, b] = 1
    np.maximum(m, m.T, out=m)            # symmetrize
    idx = np.arange(n)
    m[idx, idx] += 1                     # self-loops (may yield 2 on diag)
    deg = m.sum(axis=1, dtype=np.int32).astype(np.float32)

    x = np.asarray(x, dtype=np.float32)
    # [p, t, c] layout (pure relayout so DMA descriptors stay contiguous)
    xtp = np.ascontiguousarray(
        x.reshape(nt, 128, d).transpose(1, 0, 2)).reshape(128, nt * d)
    w = np.ascontiguousarray(np.asarray(weight, dtype=np.float32))
    bias = np.ascontiguousarray(
        np.asarray(bias, dtype=np.float32)).reshape(1, -1)
    degcol = np.ascontiguousarray(deg.reshape(nt, 128).T)

    in_maps = []
    for dv in range(ndev):
        md = m[dv * nsh:(dv + 1) * nsh]                    # [nsh, n] {0,1,2}
        # ablk[p, half, t, li'] = A[dv*nsh + half*512 + li', t*128 + p]
        ab = _F8LUT[md.reshape(2, nsh // 2, nt, 128).transpose(3, 0, 2, 1)]
        ab = np.ascontiguousarray(ab.reshape(128, nt * nsh)).view(
            ml_dtypes.float8_e4m3fn)
        in_maps.append({
            "xt": xtp, "w": w, "b": bias,
            "ablk": ab,
            "deg": degcol,
            "degmy": np.ascontiguousarray(
                deg[dv * nsh:(dv + 1) * nsh].reshape(nl, 128).T),
        })
    return in_maps


_prog_cache = {}


def _get_program():
    key = (N, D, NDEV)
    if key not in _prog_cache:
        _prog_cache[key] = _build_program()
    return _prog_cache[key]


last_results = None
TRACE = False


def kernel(x, edge_index, weight, bias):
    global last_results
    in_maps = _host_prep(x, edge_index, weight, bias)
    nc = _get_program()
    res = bass_utils.run_bass_kernel_spmd(
        nc, in_maps, core_ids=list(range(NDEV)), trace=TRACE)
    last_results = res
    out = np.concatenate([res.results[i]["out"] for i in range(NDEV)], axis=0)
    return out.astype(np.float32)
